# revision 49
# baseline (speedup 1.0000x reference)
"""Trainium2 Bass kernel for differentiable belief propagation (HMM forward-backward).

Full inputs: unary_logits (16, 4096, 128) f32, log_trans (128, 128) f32.
Output: log-marginals log_softmax(alpha+beta) of shape (16, 4096, 128) f32.

Strategy: data-parallel over batch (2 batch elements per core, 8 cores).
Per core the recursion runs in exp space with an fp16 data path:
    P = row-softmax(log_trans)                 (C x C stochastic matrix)
    eu_t = exp(u_t - 1/2)                      (drift-centered observation)
    f_t = eu_t * (P^T f_{t-1}),  f_0 = eu_0    (forward)
    G_t = P h_{t+1},  h_t = eu_t * G_t,  h_{T-1} = eu_{T-1}, G_{T-1} := 1
    w_t = f_t * G_t   (per-(b,t) positive scales cancel after normalization)
    out_t = log(w_t / S_chunk(t))
P is strictly positive so the recursion contracts projectively (~0.2/step);
the T axis splits into NCH chunks of L steps scanned in parallel, each
seeded HALO steps early.  All arrays are SET-MAJOR [C, set, b, chunk]
(t = chunk*L + set) so every scan/combine slice is contiguous.
Forward round j completes set j; backward round j completes G-set L-1-j.
From round L/2 the combine (w-mult, transpose to chunk-major, Ln, DMA out)
of the finished pair (j, L-1-j) is interleaved into the scan; G of
late-consumed sets is read straight out of PSUM, early-consumed sets are
copied once.  sum_j w_t[j] is t-invariant within a chunk (HMM evidence
invariant), so the normalizer is one reduce of one transposed set, and in
chunk-major layout it is a per-partition activation scale: out =
Ln(wT * (1/S)).  Validated error ~1e-3 vs the 2e-2 gate.
"""

import numpy as np
from contextlib import ExitStack

import concourse.bass as bass
import concourse.bacc as bacc
import concourse.mybir as mybir
from concourse import tile, masks
from concourse.bass_utils import run_bass_kernel_spmd

F32 = mybir.dt.float32
F16 = mybir.dt.float16
B, T, C = 16, 4096, 128
NCORES = 8
BLOC = B // NCORES  # batch elements per core
NCH = 256           # time chunks scanned in parallel
HALO = 4            # burn-in steps per chunk

_ALU = mybir.AluOpType
_ACT = mybir.ActivationFunctionType
_AX = mybir.AxisListType


def _build_program(t_len: int = T, bloc: int = BLOC, nch: int = NCH,
                   halo: int = HALO, reps: int = 1):
    nc = bacc.Bacc(
        "TRN2",
        target_bir_lowering=False,
        debug=False,
        num_devices=NCORES,
    )
    u = nc.dram_tensor("u", (bloc, t_len, C), F32, kind="ExternalInput").ap()
    lt = nc.dram_tensor("lt", (C, C), F32, kind="ExternalInput").ap()
    out = nc.dram_tensor("out", (bloc, t_len, C), F32, kind="ExternalOutput").ap()

    with tile.TileContext(nc) as tc:
        for r in range(reps):
            with ExitStack() as ctx:
                _body(ctx, tc, nc, u, lt, out, t_len, bloc, nch, halo, rep=r)
    nc.compile()
    return nc


def _body(ctx, tc, nc, u, lt, out, t_len, bloc, nch, halo, rep=0):
    L = t_len // nch
    assert L * nch == t_len and halo <= L and L % 2 == 0
    NW = bloc * nch              # scan matmul width
    NB = nch // C                # 128-chunk blocks per batch elem
    Q = bloc * NB                # transposed [C,C] quadrants per set
    half = L // 2                # first combined round; S reference set

    cpool = ctx.enter_context(tc.tile_pool(name=f"const{rep}", bufs=1))
    bigpool = ctx.enter_context(tc.tile_pool(name=f"big{rep}", bufs=1))
    stpool = ctx.enter_context(tc.tile_pool(name=f"stage{rep}", bufs=6))
    scrpool = ctx.enter_context(tc.tile_pool(name=f"scr{rep}", bufs=2))
    smpool = ctx.enter_context(tc.tile_pool(name=f"small{rep}", bufs=2))

    ident = cpool.tile([C, C], F32)
    masks.make_identity(nc, ident[:])
    ident16 = cpool.tile([C, C], F16)
    nc.vector.tensor_copy(ident16[:], ident[:])
    neg_half = cpool.tile([C, 1], F32)
    nc.vector.memset(neg_half[:], -0.5)
    ones16 = cpool.tile([C, C], F16)
    nc.vector.memset(ones16[:], 1.0)


    # ---- persistent set-major arrays: X[:, s, b, c] holds t = c*L + s ----
    euT = bigpool.tile([C, L, bloc, nch], F16)
    Farr = bigpool.tile([C, L, bloc, nch], F16)
    # G sets half+1..L-1 are produced L-1-2s rounds before use; keep them.
    Garr = bigpool.tile([C, half - 1, bloc, nch], F16)

    # ---- P = row-softmax(lt) in f32, cast to fp16, and its transpose ----
    with tc.tile_pool(name=f"pprep{rep}", bufs=1) as ppool, \
         tc.tile_pool(name=f"ps_pp{rep}", bufs=1, space="PSUM") as pps:
        lt_sb = ppool.tile([C, C], F32)
        nc.scalar.dma_start(out=lt_sb[:], in_=lt)
        maxv = ppool.tile([C, 1], F32)
        nc.vector.tensor_reduce(maxv[:], lt_sb[:], axis=_AX.X, op=_ALU.max)
        negmax = ppool.tile([C, 1], F32)
        nc.vector.tensor_scalar_mul(negmax[:], maxv[:], -1.0)
        pe_un = ppool.tile([C, C], F32)
        nc.scalar.activation(pe_un[:], lt_sb[:], _ACT.Exp, bias=negmax[:])
        ssum = ppool.tile([C, 1], F32)
        nc.vector.tensor_reduce(ssum[:], pe_un[:], axis=_AX.X, op=_ALU.add)
        rsum = ppool.tile([C, 1], F32)
        nc.vector.reciprocal(rsum[:], ssum[:])
        P16 = cpool.tile([C, C], F16)
        nc.vector.tensor_scalar_mul(P16[:], pe_un[:], rsum[:])
        pt_ps = pps.tile([C, C], F16, tag="ptr")
        nc.tensor.transpose(pt_ps[:], P16[:], ident16[:])
        PT16 = cpool.tile([C, C], F16)
        nc.scalar.copy(PT16[:], pt_ps[:])

    # ---- phase 0 + burn-in, emitted per batch element: b=0's burn-in
    # chains dispatch as soon as b=0's euT lands, under b=1's input DMA ----
    DGRP = 8                    # (C,128) t-blocks per input DMA / PSUM group
    dma_alt = 0
    hsc = [scrpool.tile([C, bloc, nch], F16, tag=f"hsc{k}",
                        name=f"hsc{k}") for k in range(2)]
    with tc.tile_pool(name=f"ps_tr{rep}", bufs=2, space="PSUM") as ptr0, \
         tc.tile_pool(name=f"ps_bi{rep}", bufs=1, space="PSUM") as pbi:
        for b in range(bloc):
            for d0 in range(0, t_len, C * DGRP):
                stage = stpool.tile([C, DGRP, C], F32, tag="ustage")
                dma_alt += 1
                nc.sync.dma_start(
                    out=stage[:],
                    in_=u[b, d0 : d0 + C * DGRP, :].rearrange(
                        "(blk p) j -> p blk j", p=C
                    ),
                )
                eu_st = stpool.tile([C, DGRP, C], F16, tag="eust")
                nc.scalar.activation(
                    eu_st[:].rearrange("p blk j -> p (blk j)"),
                    stage[:].rearrange("p blk j -> p (blk j)"),
                    _ACT.Exp, bias=neg_half[:],
                )
                trg = ptr0.tile([C, DGRP, C], F16, tag="trg")
                idp = ident16[:].rearrange(
                    "p (ch s) -> p s ch", ch=DGRP)
                for i in range(DGRP):
                    nc.tensor.transpose(
                        trg[:, i, :], eu_st[:, i, :], idp
                    )
                # block bl covers chunks c0+8*bl..+7; in-block t = 16*ch + s
                c0 = d0 // L
                dst = euT[:, :, b, c0 : c0 + DGRP * C // L].rearrange(
                    "p s (bl ch) -> p s bl ch", bl=DGRP)
                srcv = trg[:].rearrange("p bl (s ch) -> p s bl ch", ch=DGRP)
                nc.vector.tensor_copy(dst, srcv)
            nc.vector.tensor_copy(
                Farr[:, 0, b : b + 1, 0:1], euT[:, 0, b : b + 1, 0:1])
            if True:
                fst = bst = None
                for i in range(1, halo):
                    # fwd: state col c-1 ~ f at t = cL-halo+i (chunks 1..)
                    s = L - halo + i
                    if fst is None:
                        seed = scrpool.tile([C, 1, nch - 1], F16,
                                            tag=f"fseed{b}", name=f"fsee{b}")
                        nc.vector.tensor_copy(
                            seed[:], euT[:, s - 1, b : b + 1, 0 : nch - 1])
                        rhs = seed[:]
                    else:
                        rhs = fst[:]
                    ps = pbi.tile([C, 1, nch - 1], F32, tag="pf",
                                  name=f"pf{b}_{i}")
                    nc.tensor.matmul(ps[:], lhsT=P16[:], rhs=rhs)
                    fst = scrpool.tile([C, 1, nch - 1], F16, tag=f"fscr{b}",
                                       name=f"fscr{b}_{i}")
                    nc.vector.tensor_tensor(
                        fst[:], ps[:], euT[:, s, b : b + 1, 0 : nch - 1],
                        op=_ALU.mult)
                    # bwd: state col c ~ h at t = (c+1)L-1+halo-i (c<=n-2)
                    s = halo - 1 - i
                    if bst is None:
                        seed = scrpool.tile([C, 1, nch - 1], F16,
                                            tag=f"bseed{b}", name=f"bsee{b}")
                        nc.vector.tensor_copy(
                            seed[:], euT[:, s + 1, b : b + 1, 1:nch])
                        rhs = seed[:]
                    else:
                        rhs = bst[:]
                    ps = pbi.tile([C, 1, nch - 1], F32, tag="pb",
                                  name=f"pb{b}_{i}")
                    nc.tensor.matmul(ps[:], lhsT=PT16[:], rhs=rhs)
                    bst = scrpool.tile([C, 1, nch - 1], F16, tag=f"bscr{b}",
                                       name=f"bscr{b}_{i}")
                    nc.vector.tensor_tensor(
                        bst[:], ps[:], euT[:, s, b : b + 1, 1:nch],
                        op=_ALU.mult)
                # round 0 for this b
                ps = pbi.tile([C, 1, nch - 1], F32, tag="pf",
                              name=f"pf{b}_r0")
                nc.tensor.matmul(ps[:], lhsT=P16[:], rhs=fst[:])
                nc.vector.tensor_tensor(
                    Farr[:, 0, b : b + 1, 1:nch], ps[:],
                    euT[:, 0, b : b + 1, 1:nch], op=_ALU.mult)
                ps = pbi.tile([C, 1, nch - 1], F32, tag="pb",
                              name=f"pb{b}_r0")
                nc.tensor.matmul(ps[:], lhsT=PT16[:], rhs=bst[:])
                nc.scalar.copy(
                    Garr[:, half - 2, b : b + 1, 0 : nch - 1], ps[:])
                nc.vector.tensor_tensor(
                    hsc[0][:, b : b + 1, 0 : nch - 1], ps[:],
                    euT[:, L - 1, b : b + 1, 0 : nch - 1], op=_ALU.mult)
                nc.vector.tensor_copy(
                    hsc[0][:, b : b + 1, nch - 1 : nch],
                    euT[:, L - 1, b : b + 1, nch - 1 : nch])
    # G_{T-1} := 1
    nc.vector.memset(Garr[:, half - 2, :, nch - 1], 1.0)

    # main rounds j=1..L-1; combine pair (j, L-1-j) from round j>=half
    ps_of_set = {}
    rS_col = None
    with tc.tile_pool(name=f"ps_mm{rep}", bufs=2, space="PSUM") as pmm, \
         tc.tile_pool(name=f"ps_c{rep}", bufs=4, space="PSUM") as ptrc:
        for j in range(1, L):
            ps = pmm.tile([C, bloc, nch], F32, tag="psf")
            nc.tensor.matmul(ps[:], lhsT=P16[:], rhs=Farr[:, j - 1])
            nc.vector.tensor_tensor(
                Farr[:, j], ps[:], euT[:, j], op=_ALU.mult)
            ps = pmm.tile([C, bloc, nch], F32, tag="psb")
            nc.tensor.matmul(ps[:], lhsT=PT16[:], rhs=hsc[(j - 1) % 2][:])
            sG = L - 1 - j
            ps_of_set[sG] = ps
            if sG > half:
                nc.scalar.copy(Garr[:, sG - half - 1], ps[:])
            if j < L - 1:
                nc.vector.tensor_tensor(
                    hsc[j % 2][:], ps[:], euT[:, sG], op=_ALU.mult)

            if j < half:
                continue
            # ---- combine pair (sa=j from SBUF/held-PSUM, sb=L-1-j) ----
            sa, sb = j, L - 1 - j
            wa = stpool.tile([C, NW], F16, tag="wa")
            aeng = nc.gpsimd if sa > half else nc.vector
            asrc = Garr[:, sa - half - 1] if sa > half else ps_of_set[sa]
            aeng.tensor_tensor(
                wa[:], Farr[:, sa].rearrange("p b c -> p (b c)"),
                asrc[:].rearrange("p b c -> p (b c)"), op=_ALU.mult)
            wb = stpool.tile([C, NW], F16, tag="wb")
            nc.vector.tensor_tensor(
                wb[:], Farr[:, sb].rearrange("p b c -> p (b c)"),
                ps_of_set[sb][:].rearrange("p b c -> p (b c)"), op=_ALU.mult)
            wTr = ptrc.tile([C, 2, Q, C], F16, tag="wtr")
            for q in range(Q):
                nc.tensor.transpose(
                    wTr[:, 0, q, :], wa[:, q * C : (q + 1) * C], ident16[:])
                nc.tensor.transpose(
                    wTr[:, 1, q, :], wb[:, q * C : (q + 1) * C], ident16[:])
            og = stpool.tile([C, 2, Q, C], F32, tag="og")
            if j == half:
                # normalizer: S per chunk from set `half` (chunk-major rows)
                S_col = smpool.tile([C, Q], F32, tag="scol")
                nc.vector.tensor_reduce(
                    S_col[:], wTr[:, 0], axis=_AX.X, op=_ALU.add)
                rS_col = smpool.tile([C, Q], F32, tag="rscol")
                nc.vector.reciprocal(rS_col[:], S_col[:])
            for q in range(Q):
                nc.scalar.activation(
                    og[:, :, q, :], wTr[:, :, q, :], _ACT.Ln,
                    scale=rS_col[:, q : q + 1])
            for si, s in ((0, sa), (1, sb)):
                nc.sync.dma_start(
                    out=out[:, s::L, :].rearrange(
                        "b (m p) j -> p b m j", p=C),
                    in_=og[:, si].rearrange("p (b m) j -> p b m j", b=bloc),
                )


_cached_nc = {}


def _get_program(t_len=T, bloc=BLOC):
    key = (t_len, bloc)
    if key not in _cached_nc:
        _cached_nc[key] = _build_program(t_len, bloc)
    return _cached_nc[key]


def kernel(unary_logits: np.ndarray, log_trans: np.ndarray) -> np.ndarray:
    u = np.ascontiguousarray(unary_logits, dtype=np.float32)
    lt = np.ascontiguousarray(log_trans, dtype=np.float32)
    b_all, t_len, c = u.shape
    bloc = b_all // NCORES
    nc = _get_program(t_len, bloc)
    in_maps = [
        {"u": u[i * bloc : (i + 1) * bloc], "lt": lt} for i in range(NCORES)
    ]
    res = run_bass_kernel_spmd(nc, in_maps, list(range(NCORES)))
    outs = [res.results[i]["out"] for i in range(NCORES)]
    return np.concatenate(outs, axis=0)
